# revision 3
# baseline (speedup 1.0000x reference)
"""Trainium2 Bass kernel for edge_conv2d128 + maxpool (nn_DEDCGCNEE).

Math: out[b,c,i,j] = maxpool2x2(sum_f |conv2d(sum_ch x, sobel_f)|),
broadcast over the 128 channels.

Decomposition used:
  s = sum over channels of x                       (PE ones-matmul, fp32)
  A = conv_v(s,[1,2,1]),  B = conv_v(s,[1,0,-1]),  C = conv_v(s,[1,1,1])
  e0 = conv_h(A,[-1,0,1]); e1 = conv_h(B,[1,2,1])
  P = conv_h(C,[1,0,-1]);  Q = conv_h(B,[1,1,1])
  edge = |e0| + |e1| + 2*max(|P|,|Q|)    (since |e2|+|e3| = |P+Q|+|P-Q|)
  out = maxpool2x2(edge) broadcast over channels.

Sharding: pure data parallel over (batch 2) x (H/4 slices) = 8 cores.
Each core gets x-shard [128, 130, 512] (1-row halo each side, zero at
global edges) and produces y [128, 64, 256].

Per-core pipeline, split into 4 quarter-slices of 32 output rows so the
PE channel-sum of quarter k+1 overlaps the DVE conv + output DMA of
quarter k:
  pass 1: chunked DMA of x (ch on partitions), PE matmul with ones
          lhsT -> s rows in PSUM [1,512], ACT copy -> SBUF, DMA to a
          per-quarter DRAM scratch sq [34, 512] (2-row overlap between
          quarters is recomputed; +5% PE work, clean dependencies).
  pass 2: re-load sq with partition p = 2q+h (q=pooled row, h=w half),
          free dims (rowslot 4, w 258 incl halo); separable conv +
          abs/max on DVE; maxpool via strided max; broadcast DMAs
          (0-stride channel dim) write all 128 output channels.
"""
import sys

for _p in ("/opt/trn_rl_repo", "/opt/pypackages"):
    if _p not in sys.path:
        sys.path.insert(0, _p)

import numpy as np
import bass_rust
import concourse.bacc as bacc
import concourse.mybir as mybir
import concourse.tile as tile
from concourse.bass_utils import run_bass_kernel_spmd

N_CORES = 8
B, C, H, W = 2, 128, 512, 512
SLICE = H // 4          # 128 pre-pool rows per core
SROWS = SLICE + 2       # 130 rows of s incl halo
HO_SL, WO = SLICE // 2, W // 2   # per-core out: [C, 64, 256]
NQ = 4                  # quarter-slices per core
QROWS = SLICE // NQ + 2  # 34 s-rows per quarter (incl halo)

F32 = mybir.dt.float32
Alu = mybir.AluOpType


def make_pools(tc):
    import contextlib
    es = contextlib.ExitStack()
    pools = {
        "const": es.enter_context(tc.tile_pool(name="const", bufs=1)),
        "sdram": es.enter_context(
            tc.tile_pool(name="sdram", bufs=2, space="DRAM")),
        "p1x": es.enter_context(tc.tile_pool(name="p1x", bufs=2)),
        "psum": es.enter_context(tc.tile_pool(name="psum", bufs=2,
                                              space="PSUM")),
        "sst": es.enter_context(tc.tile_pool(name="sst", bufs=4)),
        "p2u": es.enter_context(tc.tile_pool(name="p2u", bufs=2)),
        "p2c": es.enter_context(tc.tile_pool(name="p2c", bufs=2)),
    }
    return es, pools


def emit_body(nc, pools, x, y, ones):
    """Emit one full per-core computation: x [C,130,W] -> y [C,64,WO]."""
    for k in range(NQ):
        r0 = k * (SLICE // NQ)   # x-shard row of this quarter's first s row
        # ---- pass 1: channel-sum of 34 rows -> sq [34, 512] ----
        sq = pools["sdram"].tile([QROWS, W], F32, tag="sq")
        half = QROWS // 2  # 17
        for hh in range(2):
            rt = pools["p1x"].tile([128, half * W], F32, tag="rt")
            nc.sync.dma_start(
                out=rt[:],
                in_=x[:, r0 + hh * half:r0 + (hh + 1) * half, :].rearrange(
                    "c r w -> c (r w)"))
            g0 = 0
            while g0 < half:
                g = min(4, half - g0)
                ps = pools["psum"].tile([1, 2048], F32, tag="ps")
                for j in range(g):
                    nc.tensor.matmul(
                        ps[0:1, j * 512:(j + 1) * 512], ones[:],
                        rt[:, (g0 + j) * W:(g0 + j) * W + 512])
                st = pools["sst"].tile([1, 2048], F32, tag="st")
                nc.scalar.copy(out=st[0:1, 0:g * 512],
                               in_=ps[0:1, 0:g * 512])
                nc.sync.dma_start(
                    out=sq[hh * half + g0:hh * half + g0 + g, :].rearrange(
                        "r w -> (r w)").unsqueeze(0),
                    in_=st[0:1, 0:g * 512])
                g0 += g

        # ---- pass 2: conv + pool + broadcast write for this quarter ----
        s_t = sq[:].tensor
        NP = 2 * (SLICE // NQ // 2)  # 32 local partitions (p' = 2q'+h)
        U = pools["p2u"].tile([NP, 4, 258], F32, tag="U")
        nc.vector.memset(U[:], 0.0)
        # even local partitions (w half 0): U[2q',slot,wi]=sq[2q'+slot,wi-1]
        nc.sync.dma_start(
            out=U[0:NP:2, :, 1:258],
            in_=bass_rust.AP(s_t, 0, [[2 * W, NP // 2], [W, 4], [1, 257]]))
        # odd local partitions (w half 1): U[2q'+1,slot,wi]=sq[2q'+slot,255+wi]
        nc.sync.dma_start(
            out=U[1:NP:2, :, 0:257],
            in_=bass_rust.AP(s_t, 255, [[2 * W, NP // 2], [W, 4], [1, 257]]))

        T0, T1, T2 = U[:, 0:2, :], U[:, 1:3, :], U[:, 2:4, :]
        P2 = pools["p2c"]
        sh = [NP, 2, 258]
        t_ = P2.tile(sh, F32, tag="t")
        nc.vector.tensor_add(out=t_[:], in0=T0, in1=T2)
        Bv = P2.tile(sh, F32, tag="B")
        nc.vector.tensor_sub(out=Bv[:], in0=T0, in1=T2)
        Av = P2.tile(sh, F32, tag="A")
        nc.vector.scalar_tensor_tensor(
            out=Av[:], in0=T1, scalar=2.0, in1=t_[:],
            op0=Alu.mult, op1=Alu.add)
        Cv = P2.tile(sh, F32, tag="Cv")
        nc.vector.tensor_add(out=Cv[:], in0=t_[:], in1=T1)

        sho = [NP, 2, 256]
        e0 = P2.tile(sho, F32, tag="e0")
        nc.vector.tensor_sub(out=e0[:], in0=Av[:, :, 2:258],
                             in1=Av[:, :, 0:256])
        tB = P2.tile(sho, F32, tag="tB")
        nc.vector.tensor_add(out=tB[:], in0=Bv[:, :, 2:258],
                             in1=Bv[:, :, 0:256])
        e1 = P2.tile(sho, F32, tag="e1")
        nc.vector.scalar_tensor_tensor(
            out=e1[:], in0=Bv[:, :, 1:257], scalar=2.0, in1=tB[:],
            op0=Alu.mult, op1=Alu.add)
        Pv = P2.tile(sho, F32, tag="P")
        nc.vector.tensor_sub(out=Pv[:], in0=Cv[:, :, 2:258],
                             in1=Cv[:, :, 0:256])
        Qv = P2.tile(sho, F32, tag="Q")
        nc.vector.tensor_add(out=Qv[:], in0=tB[:], in1=Bv[:, :, 1:257])
        # abs via (x * -1) max x; max(|P|,|Q|) = max(max(P,Q), -min(P,Q))
        a0 = P2.tile(sho, F32, tag="a0")
        nc.vector.scalar_tensor_tensor(
            out=a0[:], in0=e0[:], scalar=-1.0, in1=e0[:],
            op0=Alu.mult, op1=Alu.max)
        a1 = P2.tile(sho, F32, tag="a1")
        nc.vector.scalar_tensor_tensor(
            out=a1[:], in0=e1[:], scalar=-1.0, in1=e1[:],
            op0=Alu.mult, op1=Alu.max)
        Ev = P2.tile(sho, F32, tag="E")
        nc.vector.tensor_add(out=Ev[:], in0=a0[:], in1=a1[:])
        uv = P2.tile(sho, F32, tag="u")
        nc.vector.tensor_max(out=uv[:], in0=Pv[:], in1=Qv[:])
        vv = P2.tile(sho, F32, tag="v")
        nc.vector.tensor_tensor(out=vv[:], in0=Pv[:], in1=Qv[:], op=Alu.min)
        Mv = P2.tile(sho, F32, tag="M")
        nc.vector.scalar_tensor_tensor(
            out=Mv[:], in0=vv[:], scalar=-1.0, in1=uv[:],
            op0=Alu.mult, op1=Alu.max)
        E2 = P2.tile(sho, F32, tag="E2")
        nc.vector.scalar_tensor_tensor(
            out=E2[:], in0=Mv[:], scalar=2.0, in1=Ev[:],
            op0=Alu.mult, op1=Alu.add)
        wp = P2.tile([NP, 2, 128], F32, tag="wp")
        nc.vector.tensor_max(out=wp[:], in0=E2[:, :, 0:256:2],
                             in1=E2[:, :, 1:256:2])
        m = P2.tile([NP, 128], F32, tag="m")
        nc.vector.tensor_max(out=m[:], in0=wp[:, 0, :], in1=wp[:, 1, :])

        # broadcast write: y[ch, q, wo] flat = ch*16384 + p*128 + jj,
        # global p = 32k + p_local
        y_t = y[:].tensor
        CG = 32  # channels per DMA
        for g in range(C // CG):
            nc.sync.dma_start(
                out=bass_rust.AP(y_t, g * CG * 16384 + (NP * k) * 128,
                                 [[128, NP], [16384, CG], [1, 128]]),
                in_=m[:].unsqueeze(1).broadcast_to([NP, CG, 128]))


def _build_nc():
    nc = bacc.Bacc("TRN2", target_bir_lowering=False, debug=False,
                   num_devices=N_CORES)
    x = nc.dram_tensor("x", [C, SROWS, W], F32, kind="ExternalInput")
    y = nc.dram_tensor("y", [C, HO_SL, WO], F32, kind="ExternalOutput")
    with tile.TileContext(nc) as tc:
        es, pools = make_pools(tc)
        with es:
            ones = pools["const"].tile([128, 1], F32)
            nc.vector.memset(ones[:], 1.0)
            emit_body(nc, pools, x, y, ones)
    nc.compile()
    return nc


_NC_CACHE = None


def _get_nc():
    global _NC_CACHE
    if _NC_CACHE is None:
        _NC_CACHE = _build_nc()
    return _NC_CACHE


def _shard_inputs(x):
    in_maps = []
    for c in range(N_CORES):
        b, sl = c // 4, c % 4
        lo, hi = sl * SLICE - 1, sl * SLICE + SLICE + 1
        shard = np.zeros((C, SROWS, W), np.float32)
        slo, shi = max(lo, 0), min(hi, H)
        shard[:, slo - lo:slo - lo + (shi - slo), :] = x[b, :, slo:shi, :]
        in_maps.append({"x": np.ascontiguousarray(shard)})
    return in_maps


def kernel(x):
    x = np.asarray(x)
    assert x.shape == (B, C, H, W), x.shape
    nc = _get_nc()
    res = run_bass_kernel_spmd(nc, _shard_inputs(x), list(range(N_CORES)))
    out = np.empty((B, C, H // 2, W // 2), np.float32)
    for c in range(N_CORES):
        b, sl = c // 4, c % 4
        out[b, :, sl * HO_SL:(sl + 1) * HO_SL, :] = res.results[c]["y"]
    return out


if __name__ == "__main__":
    rng = np.random.default_rng(0)
    xv = rng.standard_normal((B, C, H, W), dtype=np.float32)
    out = kernel(x=xv)
    print("kernel ran, out shape", out.shape)
